# revision 5
# baseline (speedup 1.0000x reference)
"""Trainium2 Bass kernel for a GNN message-passing layer (GCL) — v3.

reference:
    m   = relu(concat(h[row], h[col]) @ edge_w + edge_b)       # [E, H]
    agg = segment_sum(m, row, N)                               # [N, H]
    out = relu(concat(h, agg) @ node_w + node_b)               # [N, H]

Single col-side gather (256B/edge); the row side is reconstructed on PE:
per 128-edge chunk, ohT = PE-transpose(onehot(row_local)), then
rv = ohT.T @ A_window and colg is accumulated into the same PSUM via an
identity-stationary matmul; ACT relu drains PSUM->SBUF. Aggregation is
m.T @ onehot accumulated across a window's 4 col-chunk segments in one
PSUM group (w-major segment order), flushed once per window, with the
node MLP for that window folded into the same pipeline. Work is batched
in global 8-chunk tiles; a 3-segment software-pipeline skew keeps PE
from stalling on DVE/ACT results.
"""

import math
import numpy as np
import ml_dtypes

import concourse.bass as bass
import concourse.bacc as bacc
import concourse.tile as tile
from concourse import mybir
from concourse.tile import TileContext
from concourse.library_config import mlp as mlp_library

BF16 = mybir.dt.bfloat16
F32 = mybir.dt.float32
I16 = mybir.dt.int16
NP_BF16 = ml_dtypes.bfloat16


class Cfg:
    def __init__(self, n_nodes, n_cores=8):
        self.n_swdge_queues = 4
        self.col_sort = True
        self.N = n_nodes
        self.n_cores = n_cores
        self.NPC = int(math.ceil(n_nodes / n_cores / 128)) * 128
        self.NP = self.NPC * n_cores
        self.W = self.NPC // 128          # windows per core
        self.C = 4                        # col chunks
        assert self.NP % self.C == 0
        self.CHUNK = self.NP // self.C
        assert self.CHUNK <= 32767, "int16 gather index limit"
        self.SEG = None  # set from data

    def stripe(self, total):
        for cand in (8192, 6272, 4096, 3136, 2048, 1792, 1568, 1024, 896,
                     784, 512, 448, 256, 128):
            if cand <= total and total % cand == 0:
                return cand
        raise AssertionError(total)


def build_kernel(cfg, phases=(0, 1, 2), p1_level=4):
    SEG = cfg.SEG
    assert SEG is not None and SEG % 128 == 0
    S = cfg.C * cfg.W                      # number of segments
    EP = S * SEG                           # padded edges per core
    JPS = SEG // 128                       # 128-chunks per segment
    assert JPS >= 7, "pipeline skew assumes batches trail by <1 segment"
    NCHUNK = S * JPS                       # total chunks
    NB = (NCHUNK + 7) // 8                 # 8-chunk batches
    SINGLE_PACKET = (2 * SEG // 16 + 1) <= 64
    GSEG = 32                              # segments per cidx load
    ILEN = GSEG * SEG // 16

    NSWQ = cfg.n_swdge_queues
    nc = bacc.Bacc("TRN2", target_bir_lowering=False, debug=False,
                   num_swdge_queues=NSWQ)

    # ---- DRAM I/O ----
    hTa_d = nc.dram_tensor("hTa", [65, cfg.NP], BF16, kind="ExternalInput")
    hTown_d = nc.dram_tensor("hTown", [65, cfg.NPC], BF16, kind="ExternalInput")
    waug_d = nc.dram_tensor("waug", [65, 128], BF16, kind="ExternalInput")
    nw1_d = nc.dram_tensor("nw1", [64, 64], BF16, kind="ExternalInput")
    nw2a_d = nc.dram_tensor("nw2a", [65, 64], F32, kind="ExternalInput")
    iota_d = nc.dram_tensor("iota", [128, 128], BF16, kind="ExternalInput")
    ident_d = nc.dram_tensor("ident", [128, 128], BF16, kind="ExternalInput")
    colidx_d = nc.dram_tensor("colidx", [128, EP // 16], I16, kind="ExternalInput")
    rl_d = nc.dram_tensor("rl", [128, EP // 128], BF16, kind="ExternalInput")
    AB_ds = [nc.dram_tensor(f"AB{c}", [cfg.CHUNK, 128], BF16)
             for c in range(cfg.C)]
    out_d = nc.dram_tensor("out", [cfg.NPC, 64], F32, kind="ExternalOutput")

    with TileContext(nc) as tc:
        nc.gpsimd.load_library(mlp_library)

        with tc.tile_pool(name="const", bufs=1) as cpool:
            waug_sb = cpool.tile([65, 128], BF16)
            nc.sync.dma_start(out=waug_sb[:], in_=waug_d[:])
            iota_sb = cpool.tile([128, 128], BF16)
            nc.sync.dma_start(out=iota_sb[:], in_=iota_d[:])
            ident_sb = cpool.tile([128, 128], BF16)
            nc.sync.dma_start(out=ident_sb[:], in_=ident_d[:])
            nw1_sb = cpool.tile([64, 64], BF16)
            nc.sync.dma_start(out=nw1_sb[:], in_=nw1_d[:])
            nw2a_sb = cpool.tile([65, 64], F32)
            nc.sync.dma_start(out=nw2a_sb[:], in_=nw2a_d[:])

            hTown_sb = cpool.tile([65, cfg.NPC], BF16)
            nc.sync.dma_start(out=hTown_sb[:], in_=hTown_d[:])

            aown_sb = cpool.tile([128, cfg.W, 64], BF16)
            arena = cpool.tile([65, cfg.NPC], F32)
            nc.vector.memset(arena[64:65, :], 1.0)

            # ---- Phase 0a: Aown into SBUF ----
            if 0 in phases:
                with tc.tile_pool(name="p0aps", bufs=4, space="PSUM") as p0aps:
                    for wb in range(cfg.W // 2):
                        ps = p0aps.tile([128, 2, 128], F32)
                        for i in range(2):
                            w = wb * 2 + i
                            nc.tensor.matmul(
                                out=ps[:, i, :],
                                lhsT=hTown_sb[:, w * 128:(w + 1) * 128],
                                rhs=waug_sb[:], start=True, stop=True)
                        if wb % 2 == 0:
                            nc.vector.tensor_copy(
                                out=aown_sb[:, wb * 2:wb * 2 + 2, :],
                                in_=ps[:, :, 0:64])
                        else:
                            nc.scalar.activation(
                                out=aown_sb[:, wb * 2:wb * 2 + 2, :],
                                in_=ps[:, :, 0:64],
                                func=mybir.ActivationFunctionType.Copy)

                # ---- Phase 0b: AB table (all NP nodes) to DRAM ----
                SN = cfg.stripe(cfg.CHUNK)
                JT = SN // 128
                with tc.tile_pool(name="p0", bufs=2) as p0, \
                     tc.tile_pool(name="p0ps", bufs=4, space="PSUM") as p0ps:
                    for st in range(cfg.NP // SN):
                        hstripe = p0.tile([65, SN], BF16, tag="hstripe")
                        nc.sync.dma_start(
                            out=hstripe[:], in_=hTa_d[:, st * SN:(st + 1) * SN])
                        abst = p0.tile([128, JT, 128], BF16, tag="abst")
                        for jb in range((JT + 3) // 4):
                            n = min(4, JT - jb * 4)
                            ps = p0ps.tile([128, 4, 128], F32)
                            for i in range(n):
                                j = jb * 4 + i
                                nc.tensor.matmul(
                                    out=ps[:, i, :],
                                    lhsT=hstripe[:, j * 128:(j + 1) * 128],
                                    rhs=waug_sb[:], start=True, stop=True)
                            if jb % 2 == 0:
                                nc.vector.tensor_copy(
                                    out=abst[:, jb * 4:jb * 4 + n, :],
                                    in_=ps[:, 0:n, :])
                            else:
                                nc.scalar.activation(
                                    out=abst[:, jb * 4:jb * 4 + n, :],
                                    in_=ps[:, 0:n, :],
                                    func=mybir.ActivationFunctionType.Copy)
                        n0 = st * SN
                        dst_d, off = AB_ds[n0 // cfg.CHUNK], n0 % cfg.CHUNK
                        # hTa columns are host-permuted so node off+p*JT+j is
                        # abst[p, j]; per-partition runs are contiguous
                        nc.sync.dma_start(
                            out=dst_d[off:off + SN, :].rearrange(
                                "(p j) f -> p j f", p=128),
                            in_=abst[:])

            # ---- Phase 1 + fused phase 2: skewed pipeline over segments ----
            if 1 in phases:
              with tc.tile_pool(name="rlp", bufs=1) as rlp:
                rl_sb = rlp.tile([128, EP // 128], BF16)
                nc.sync.dma_start(out=rl_sb[:], in_=rl_d[:])

                with tc.tile_pool(name="idxp", bufs=2) as idxp, \
                     tc.tile_pool(name="gath", bufs=4) as gathp, \
                     tc.tile_pool(name="ohp", bufs=6) as ohp, \
                     tc.tile_pool(name="ohtp", bufs=4) as ohtp, \
                     tc.tile_pool(name="mp", bufs=4) as mp, \
                     tc.tile_pool(name="p2o", bufs=2) as p2o, \
                     tc.tile_pool(name="tps", bufs=2, space="PSUM") as tps, \
                     tc.tile_pool(name="rvps", bufs=2, space="PSUM") as rvps, \
                     tc.tile_pool(name="aggps", bufs=2, space="PSUM") as aggps, \
                     tc.tile_pool(name="p2ps", bufs=1, space="PSUM") as p2ps:

                    cidx_tiles = {}
                    oh_tiles = {}          # seg -> tile
                    colg_tiles = {}        # seg -> tile
                    tb_tiles = {}          # batch -> psum tile
                    oht_tiles = {}         # batch -> sbuf tile
                    rv_tiles = {}          # batch -> psum tile
                    m2_tiles = {}          # batch -> sbuf tile
                    agg_tiles = {}         # window -> psum tile
                    n_T = [0]              # chunks transposed so far
                    n_rv = [0]             # chunks rv+madd'ed so far
                    n_relu = [0]           # batches relu'ed so far
                    ost_tiles = {}

                    def load_cidx(g):
                        if g * GSEG >= S:
                            return
                        n = min(GSEG, S - g * GSEG) * SEG // 16
                        t = idxp.tile([128, ILEN], I16, tag="cidx")
                        nc.sync.dma_start(
                            out=t[:, 0:n], in_=colidx_d[:, g * ILEN:g * ILEN + n])
                        cidx_tiles[g] = t

                    def gather(s):
                        # one call per segment PAIR (w-pair order: both have
                        # the same col chunk c = (s%8)//2)
                        if s % 2 == 1:
                            return
                        g, r = divmod(s, GSEG)
                        t = gathp.tile([128, 2 * JPS, 128], BF16, tag="g")
                        nc.gpsimd.dma_gather(
                            t[:], AB_ds[(s % 8) // 2][:],
                            cidx_tiles[g][:, r * (SEG // 16):(r + 2) * (SEG // 16)],
                            2 * SEG, 2 * SEG, 128, single_packet=SINGLE_PACKET,
                            queue_num=(s // 2) % NSWQ)
                        colg_tiles[s] = t
                        colg_tiles[s + 1] = t

                    def build_oh(s):
                        t = ohp.tile([128, JPS, 128], BF16, tag="oh")
                        nc.vector.tensor_tensor(
                            out=t[:],
                            in0=rl_sb[:, s * JPS:(s + 1) * JPS]
                                .to_broadcast([128, JPS, 128]),
                            in1=iota_sb[:].rearrange("p (a b) -> p a b", a=1)
                                .to_broadcast([128, JPS, 128]),
                            op=mybir.AluOpType.is_equal)
                        oh_tiles[s] = t

                    def transpose_upto(klim):
                        # transpose chunks [n_T, klim) into batch psum tiles;
                        # drain every completed batch (alternate DVE/ACT)
                        while n_T[0] < klim:
                            k = n_T[0]
                            b = k // 8
                            if k % 8 == 0:
                                tb_tiles[b] = tps.tile([128, 8, 128], BF16,
                                                       tag="tb", name="tb")
                            s, j = divmod(k, JPS)
                            nc.tensor.transpose(
                                out=tb_tiles[b][:, k % 8, :],
                                in_=oh_tiles[s][:, j, :],
                                identity=ident_sb[:])
                            n_T[0] += 1
                            if n_T[0] % 8 == 0 or n_T[0] == NCHUNK:
                                nb_ = 8 if n_T[0] % 8 == 0 else n_T[0] % 8
                                dst = ohtp.tile([128, 8, 128], BF16, tag="ohT")
                                if b % 4 == 0:
                                    nc.vector.tensor_copy(
                                        out=dst[:, 0:nb_, :],
                                        in_=tb_tiles[b][:, 0:nb_, :])
                                else:
                                    nc.scalar.activation(
                                        out=dst[:, 0:nb_, :],
                                        in_=tb_tiles[b][:, 0:nb_, :],
                                        func=mybir.ActivationFunctionType.Copy)
                                oht_tiles[b] = dst
                                tb_tiles.pop(b)

                    def rv_madd_seg(s):
                        # rv + colg accumulate for all chunks of segment s;
                        # relu every batch that becomes fully filled
                        w = (s // 8) * 2 + (s % 2)
                        half = (s % 2) * JPS
                        for j in range(JPS):
                            k = s * JPS + j
                            b = k // 8
                            if k % 8 == 0:
                                rv_tiles[b] = rvps.tile([128, 8, 64], F32,
                                                        tag="rv", name="rv")
                            nc.tensor.matmul(
                                out=rv_tiles[b][:, k % 8, :],
                                lhsT=oht_tiles[b][:, k % 8, :],
                                rhs=aown_sb[:, w, :], start=True, stop=False)
                            nc.tensor.matmul(
                                out=rv_tiles[b][:, k % 8, :],
                                lhsT=ident_sb[:],
                                rhs=colg_tiles[s][:, half + j, 64:128],
                                start=False, stop=True)
                            n_rv[0] += 1
                        colg_tiles.pop(s)
                        while (n_relu[0] + 1) * 8 <= n_rv[0] or \
                              n_rv[0] == NCHUNK and n_relu[0] < NB:
                            b = n_relu[0]
                            nb_ = min(8, NCHUNK - b * 8)
                            dst = mp.tile([128, 8, 64], BF16, tag="m2")
                            nc.scalar.activation(
                                out=dst[:, 0:nb_, :],
                                in_=rv_tiles[b][:, 0:nb_, :],
                                func=mybir.ActivationFunctionType.Relu)
                            m2_tiles[b] = dst
                            rv_tiles.pop(b)
                            n_relu[0] += 1

                    def agg_seg(s):
                        w = (s // 8) * 2 + (s % 2)
                        c = (s % 8) // 2
                        if c == 0:
                            agg_tiles[w] = aggps.tile([64, 128], F32, tag="agg", name="agg")
                        for j in range(JPS):
                            k = s * JPS + j
                            b = k // 8
                            nc.tensor.matmul(
                                out=agg_tiles[w][:],
                                lhsT=m2_tiles[b][:, k % 8, :],
                                rhs=oh_tiles[s][:, j, :],
                                start=(c == 0 and j == 0),
                                stop=(c == cfg.C - 1 and j == JPS - 1))
                        oh_tiles.pop(s)
                        if (s * JPS + JPS) % 8 != 0:
                            pass  # m2 batches span segments; popped lazily
                        for b in [bb for bb in m2_tiles
                                  if (bb + 1) * 8 <= s * JPS + JPS]:
                            m2_tiles.pop(b)
                        if c == cfg.C - 1:
                            flush_window(w)

                    def flush_window(w):
                        nc.vector.tensor_copy(
                            out=arena[0:64, w * 128:(w + 1) * 128],
                            in_=agg_tiles[w][:])
                        agg_tiles.pop(w)
                        if 2 in phases:
                            node_mlp(w)

                    def node_mlp(w):
                        if w % 2 == 0:
                            ost_tiles[w // 2] = p2o.tile([128, 2, 64], F32,
                                                         tag="ost", name="ost")
                        ps = p2ps.tile([128, 64], F32)
                        nc.tensor.matmul(
                            out=ps[:], lhsT=hTown_sb[0:64, w * 128:(w + 1) * 128],
                            rhs=nw1_sb[:], start=True, stop=False)
                        nc.tensor.matmul(
                            out=ps[:], lhsT=arena[:, w * 128:(w + 1) * 128],
                            rhs=nw2a_sb[:], start=False, stop=True)
                        nc.scalar.activation(
                            out=ost_tiles[w // 2][:, w % 2, :], in_=ps[:],
                            func=mybir.ActivationFunctionType.Relu)
                        if w % 2 == 1:
                            # row p*2+i of the block holds node (w-1+i)*128+p;
                            # unshard_output inverts this on host
                            nc.sync.dma_start(
                                out=out_d[(w - 1) * 128:(w + 1) * 128, :]
                                    .rearrange("(p i) f -> p i f", p=128),
                                in_=ost_tiles[w // 2][:])
                            ost_tiles.pop(w // 2)

                    # ---- the skewed main loop ----
                    load_cidx(0)
                    load_cidx(1)
                    for s in range(S + 3):
                        if s < S and s % GSEG == 0 and s > 0:
                            load_cidx(s // GSEG + 1)
                        if s < S:
                            gather(s)
                        if s < S:
                            build_oh(s)
                        if p1_level >= 2:
                            if s >= 1:
                                transpose_upto(min((s - 1 + 1) * JPS, NCHUNK))
                            if s >= 2 and s - 2 < S:
                                rv_madd_seg(s - 2)
                            if p1_level >= 4 and s >= 3 and s - 3 < S:
                                agg_seg(s - 3)

            elif 2 in phases:
                # standalone phase 2 (for ablation)
                with tc.tile_pool(name="p2", bufs=2) as p2, \
                     tc.tile_pool(name="p2ps", bufs=4, space="PSUM") as p2ps:
                    for g in range(cfg.W // 2):
                        ost = p2.tile([128, 2, 64], F32, tag="ost")
                        for i in range(2):
                            w = g * 2 + i
                            ps = p2ps.tile([128, 64], F32)
                            nc.tensor.matmul(
                                out=ps[:],
                                lhsT=hTown_sb[0:64, w * 128:(w + 1) * 128],
                                rhs=nw1_sb[:], start=True, stop=False)
                            nc.tensor.matmul(
                                out=ps[:], lhsT=arena[:, w * 128:(w + 1) * 128],
                                rhs=nw2a_sb[:], start=False, stop=True)
                            nc.scalar.activation(
                                out=ost[:, i, :], in_=ps[:],
                                func=mybir.ActivationFunctionType.Relu)
                        nc.sync.dma_start(
                            out=out_d[g * 256:(g + 1) * 256, :].rearrange(
                                "(p i) f -> p i f", p=128),
                            in_=ost[:])

    nc.compile()
    return nc


# ---------------- host-side data prep ----------------

def _wrap16(a):
    x = np.ascontiguousarray(a.reshape(-1, 16).T)
    return np.tile(x, (8, 1))


def _wrap128(a):
    return np.ascontiguousarray(a.reshape(-1, 128).T)


def prep_inputs(cfg, h, edge_index, edge_w, edge_b, node_w, node_b):
    N = cfg.N
    row = np.asarray(edge_index[0])
    col = np.asarray(edge_index[1])
    h = np.asarray(h, dtype=np.float32)

    hTa = np.zeros((65, cfg.NP), NP_BF16)
    hTa[:64, :N] = h.T.astype(NP_BF16)
    hTa[64, :] = 1.0
    # permute columns so p0b's natural (p, j) write order lands nodes
    # contiguously per partition: stripe col j*128+p <- node st*SN + p*JT + j
    SN = cfg.stripe(cfg.CHUNK)
    JT = SN // 128
    hTa_p = hTa.reshape(65, cfg.NP // SN, 128, JT).swapaxes(2, 3) \
        .reshape(65, cfg.NP)

    waug = np.zeros((65, 128), NP_BF16)
    waug[:64, 0:64] = edge_w[:64].astype(NP_BF16)
    waug[:64, 64:128] = edge_w[64:].astype(NP_BF16)
    waug[64, 0:64] = edge_b.astype(NP_BF16)

    nw1 = np.ascontiguousarray(node_w[:64]).astype(NP_BF16)
    nw2a = np.concatenate([node_w[64:], node_b[None, :]],
                          axis=0).astype(np.float32)

    iota = np.tile(np.arange(128, dtype=np.float32), (128, 1)).astype(NP_BF16)
    ident = np.eye(128, dtype=np.float32).astype(NP_BF16)

    # per-core edge prep; w-major segment order: seg_id = w*C + cc
    per_core = []
    maxc = 1
    for k in range(cfg.n_cores):
        base = k * cfg.NPC
        m = (row >= base) & (row < base + cfg.NPC)
        r = (row[m] - base).astype(np.int64)
        c = col[m].astype(np.int64)
        w = r >> 7
        cc = c // cfg.CHUNK
        seg_id = (w >> 1) * 2 * cfg.C + cc * 2 + (w & 1)
        if cfg.col_sort:
            order = np.lexsort((c, seg_id))
        else:
            order = np.argsort(seg_id, kind="stable")
        r, c, seg_id = r[order], c[order], seg_id[order]
        counts = np.bincount(seg_id, minlength=cfg.C * cfg.W)
        if counts.size and r.size:
            maxc = max(maxc, int(counts.max()))
        per_core.append((r, c, seg_id, counts))
    SEG = int(math.ceil(maxc / 128.0)) * 128
    cfg.SEG = SEG
    EP = cfg.C * cfg.W * SEG

    in_maps = []
    for k in range(cfg.n_cores):
        r, c, seg_id, counts = per_core[k]
        starts = np.cumsum(counts) - counts
        intra = np.arange(r.size) - np.repeat(starts, counts)
        slots = seg_id * SEG + intra
        colidx = np.zeros(EP, np.int16)
        rl = np.full(EP, 255.0, NP_BF16)
        colidx[slots] = (c - (c // cfg.CHUNK) * cfg.CHUNK).astype(np.int16)
        rl[slots] = (r & 127).astype(NP_BF16)

        base = k * cfg.NPC
        hTown = np.ascontiguousarray(hTa[:, base:base + cfg.NPC])
        in_maps.append({
            "hTa": hTa_p,
            "hTown": hTown,
            "waug": waug,
            "nw1": nw1,
            "nw2a": nw2a,
            "iota": iota,
            "ident": ident,
            "colidx": _wrap16(colidx),
            "rl": _wrap128(rl),
        })
    return in_maps


def unshard_output(cfg, results):
    outs = []
    for res in results:
        o = np.asarray(res["out"])                      # [NPC, 64] permuted
        o = o.reshape(-1, 128, 2, 64).swapaxes(1, 2).reshape(-1, 64)
        outs.append(o)
    full = np.concatenate(outs, axis=0)
    return np.ascontiguousarray(full[:cfg.N]).astype(np.float32)


# ---------------- entry point ----------------

def kernel(h, edge_index, edge_w, edge_b, node_w, node_b):
    from concourse.bass_utils import run_bass_kernel_spmd
    cfg = Cfg(n_nodes=100000, n_cores=8)
    in_maps = prep_inputs(cfg, h, edge_index, edge_w, edge_b, node_w, node_b)
    nc = build_kernel(cfg)
    res = run_bass_kernel_spmd(nc, in_maps, core_ids=list(range(cfg.n_cores)))
    return unshard_output(cfg, res.results)
